# revision 24
# baseline (speedup 1.0000x reference)
"""Trainium2 Bass kernel for DiffusionRNNAgent sampling.

Data-parallel over 8 NeuronCores (batch 32768 -> 4096/core). Feature-major
layout on device: activations stored [features(partitions), batch(free)].
Host does transposes, schedule constants, time-MLP folding, noise pre-scaling.

mish(x) = x * tanh(softplus(x)) is computed as x * g where, with s = sigmoid(x):
    g = num/den, num = 2s - s^2 = 1 - (s-1)^2, den = 2 - num = (s-1)^2 + 1.
den lies in [1,2], so a linear seed + 2 Newton steps gives ~1e-5 accuracy.
One ACT pass (sigmoid, fused layer bias), two custom 8-slice DVE ops, one
ACT identity pass to materialize (z+b), one gpsimd multiply.
"""

import re

import numpy as np

import concourse.bass as bass
import concourse.mybir as mybir
import concourse.tile as tile
import concourse.dve_ops as dve_ops
from concourse import bacc
from concourse.dve_spec import Spec, Src0, Src1, C0, C1, C2, sq
from concourse.bass_utils import run_bass_kernel_spmd

BATCH, IN_DIM, RNN_H, N_ACT = 32768, 128, 256, 32
T_DIM, MID, N_T, MAX_ACTION = 16, 256, 10, 1.0
N_CORES = 8
BC = BATCH // N_CORES          # 4096 batch per core
import os as _os
OPC_MODE = _os.environ.get("OPC_MODE", "gpsimd")  # gpsimd | dve
N_UNITS = 4                    # units of 1024 cols
UN = BC // N_UNITS             # 1024
F32 = mybir.dt.float32


# ---------------------------------------------------------------- custom DVE ops
def _register_dve_op(name, spec, subdim=False, perf_en=None):
    for op in dve_ops.OPS:
        if op.name == name:
            return op
    probe = dve_ops.DveOp(name=name, spec=spec, subdim=subdim, uops_sha={},
                          perf_en=perf_en or {})
    dve_ops.OPS.append(probe)
    dve_ops._SUB_OPCODE_FOR_NAME[name] = dve_ops._CUSTOM_DVE_ROW_BASE + len(dve_ops.OPS) - 1
    dve_ops.CUSTOM_DVE_SPECS[name] = spec
    shas = {}
    for ver in ("v3",):
        try:
            probe.compile(ver)
            shas[ver] = probe.uops_sha.get(ver)
        except ValueError as e:
            m = re.search(rf"{ver}: ([0-9a-f]+)", str(e))
            if not m:
                raise
            shas[ver] = m.group(1)
    final = dve_ops.DveOp(name=name, spec=spec, subdim=subdim, uops_sha=shas,
                          perf_en=perf_en or {})
    dve_ops.OPS[-1] = final
    dve_ops.CUSTOM_DVE_SPECS[name] = spec
    dve_ops._COMPILE_CACHE.pop((name, "v3"), None)
    return final


def _ref_opa(in0, in1, s0, s1, imm2):
    t = in0.astype(np.float32) - s0
    w = t * t
    den = w + s0
    y0 = s1 * (imm2 - w)
    return (y0 * (imm2 - den * y0)).astype(np.float32)


def _ref_opb(in0, in1, s0, s1, imm2):
    t = in0.astype(np.float32) - s0
    w = t * t
    den = w + s0
    y2 = in1 * (imm2 - den * in1)
    return ((s0 - w) * y2).astype(np.float32)


# opA: in0 = s (sigmoid).  C0=1, C1=seed coeff, C2=2.
#   w=(s-1)^2; den=w+1; y0=C1*(2-w); y1=y0*(2-den*y0)   -> ~0.35% recip approx
_t = Src0 - C0
_w = sq(_t)
_den = _w + C0
_y0 = C1 * (C2 - _w)
MISH_RECIP_A = Spec(body=_y0 * (C2 - _den * _y0), reference=_ref_opa)

# opB: in0 = s, in1 = y1.  y2 = NR(y1); g = (1-w)*y2 = num/den
_y2 = Src1 * (C2 - _den * Src1)
MISH_RECIP_B = Spec(body=(C0 - _w) * _y2, reference=_ref_opb)


def _ref_opnr(in0, in1, s0, s1, imm2):
    t = in0.astype(np.float32) - s0
    den = t * t + s0
    return (in1 * (imm2 - den * in1)).astype(np.float32)


# opNR: pure Newton step y2 = y*(2 - den*y) for an extra refinement pass
MISH_NR_ONLY = Spec(body=_y2, reference=_ref_opnr)

_OPA = _register_dve_op("MISH_SEED_NR_ANT", MISH_RECIP_A, perf_en={"v3": True})
_OPB = _register_dve_op("MISH_NR_MUL_ANT", MISH_RECIP_B)
_OPNR = _register_dve_op("MISH_NR_ONLY_ANT", MISH_NR_ONLY)

SEED_C1 = 2.0 / (2.0 + 2.25)   # minimax linear seed for 1/d on [1,2]


# ---------------------------------------------------------------- host constants
def _cosine_betas(T, s=0.008):
    steps = T + 1
    x = np.linspace(0, steps, steps)
    ac = np.cos(((x / steps) + s) / (1 + s) * np.pi * 0.5) ** 2
    ac = ac / ac[0]
    return np.clip(1.0 - ac[1:] / ac[:-1], 0.0, 0.999)


def _diff_consts():
    betas = _cosine_betas(N_T)
    alphas = 1.0 - betas
    ac = np.cumprod(alphas)
    acp = np.concatenate([[1.0], ac[:-1]])
    post_var = betas * (1.0 - acp) / (1.0 - ac)
    return dict(
        sqrt_recip=np.sqrt(1.0 / ac),
        sqrt_recipm1=np.sqrt(1.0 / ac - 1.0),
        log_var=np.log(np.maximum(post_var, 1e-20)),
        coef1=betas * np.sqrt(acp) / (1.0 - ac),
        coef2=(1.0 - acp) * np.sqrt(alphas) / (1.0 - ac),
    )


def _mish_np(x):
    return (x * np.tanh(np.log1p(np.exp(x)))).astype(np.float32)


def _time_emb(i):
    half = T_DIM // 2
    freqs = np.exp(np.arange(half, dtype=np.float32) * (-np.log(10000.0) / (half - 1)))
    ang = np.float32(i) * freqs
    return np.concatenate([np.sin(ang), np.cos(ang)]).astype(np.float32)


_BUILT = {}


def _build_module():
    """Build + finalize the Bass module once. Returns (nc, names)."""
    if "nc" in _BUILT:
        return _BUILT["nc"]

    nc = bacc.Bacc("TRN2", target_bir_lowering=False, debug=False)

    d_in = nc.dram_tensor("inp_t", [IN_DIM, BC], F32, kind="ExternalInput")
    d_hid = nc.dram_tensor("hid_t", [2, 128, BC], F32, kind="ExternalInput")
    d_xinit = nc.dram_tensor("xinit_t", [N_ACT, BC], F32, kind="ExternalInput")
    d_noise = nc.dram_tensor("noise_t", [N_T - 1, N_ACT, BC], F32, kind="ExternalInput")
    d_fc1 = nc.dram_tensor("fc1_t", [IN_DIM, RNN_H], F32, kind="ExternalInput")
    d_wih = nc.dram_tensor("wih_t", [2, 128, 3 * RNN_H], F32, kind="ExternalInput")
    d_whh = nc.dram_tensor("whh_t", [2, 128, 3 * RNN_H], F32, kind="ExternalInput")
    # m1 weights with te-columns dropped: K chunks 32(xa) / 128(h0) / 128(h1)
    d_m1a = nc.dram_tensor("m1a_t", [32, MID], F32, kind="ExternalInput")
    d_m1b = nc.dram_tensor("m1b_t", [128, MID], F32R, kind="ExternalInput")
    d_m1c = nc.dram_tensor("m1c_t", [128, MID], F32R, kind="ExternalInput")
    d_m2 = nc.dram_tensor("m2_t", [2, 128, MID], F32R, kind="ExternalInput")
    d_m3 = nc.dram_tensor("m3_t", [2, 128, MID], F32R, kind="ExternalInput")
    d_fin = nc.dram_tensor("fin_t", [2, 128, N_ACT], F32R, kind="ExternalInput")
    d_consts = nc.dram_tensor("consts", [128, 64], F32, kind="ExternalInput")
    d_m1b32 = nc.dram_tensor("m1b32_t", [128, MID], F32, kind="ExternalInput")
    d_m1c32 = nc.dram_tensor("m1c32_t", [128, MID], F32, kind="ExternalInput")
    d_m232 = nc.dram_tensor("m232_t", [2, 128, MID], F32, kind="ExternalInput")
    d_m332 = nc.dram_tensor("m332_t", [2, 128, MID], F32, kind="ExternalInput")
    d_fin32 = nc.dram_tensor("fin32_t", [2, 128, N_ACT], F32, kind="ExternalInput")

    d_q = nc.dram_tensor("q_out", [N_ACT, BC], F32, kind="ExternalOutput")
    d_h = nc.dram_tensor("h_out", [2, 128, BC], F32, kind="ExternalOutput")

    SIG = mybir.ActivationFunctionType.Sigmoid
    TANH = mybir.ActivationFunctionType.Tanh
    RELU = mybir.ActivationFunctionType.Relu
    IDEN = mybir.ActivationFunctionType.Identity
    ADD = mybir.AluOpType.add
    MULT = mybir.AluOpType.mult
    MINOP = mybir.AluOpType.min
    MAXOP = mybir.AluOpType.max

    C = _diff_consts()
    steps = list(enumerate(reversed(range(N_T))))  # (step, i): i = 9..0

    with tile.TileContext(nc) as tc:
        with (
            tc.tile_pool(name="const", bufs=1) as constp,
            tc.tile_pool(name="persist", bufs=1) as persist,
            tc.tile_pool(name="work", bufs=2) as work,
            tc.tile_pool(name="mout", bufs=2) as moutp,
            tc.tile_pool(name="small", bufs=3) as small,
            tc.tile_pool(name="noisep", bufs=2) as noisep,
            tc.tile_pool(name="ps", bufs=3, space="PSUM") as ps,
            tc.tile_pool(name="psfin", bufs=1, space="PSUM") as psfin,
        ):
            # ---------- load weights/constants
            w_fc1 = constp.tile([IN_DIM, RNN_H], F32)
            nc.sync.dma_start(w_fc1[:], d_fc1[:])
            w_wih = [constp.tile([128, 3 * RNN_H], F32, tag=f"wih{k}") for k in range(2)]
            for k in range(2):
                nc.sync.dma_start(w_wih[k][:], d_wih[k])
            w_whh = [constp.tile([128, 3 * RNN_H], F32, tag=f"whh{k}") for k in range(2)]
            for k in range(2):
                nc.sync.dma_start(w_whh[k][:], d_whh[k])
            w_m1a = constp.tile([32, MID], F32)
            nc.sync.dma_start(w_m1a[:], d_m1a[:])
            w_m1b = constp.tile([128, MID], F32R)
            nc.sync.dma_start(w_m1b[:], d_m1b[:])
            w_m1c = constp.tile([128, MID], F32R)
            nc.sync.dma_start(w_m1c[:], d_m1c[:])
            w_m2 = [constp.tile([128, MID], F32, tag=f"m2w{k}") for k in range(2)]
            for k in range(2):
                nc.sync.dma_start(w_m2[k][:], d_m2[k])
            w_m3 = [constp.tile([128, MID], F32, tag=f"m3w{k}") for k in range(2)]
            for k in range(2):
                nc.sync.dma_start(w_m3[k][:], d_m3[k])
            w_fin = [constp.tile([128, N_ACT], F32, tag=f"finw{k}") for k in range(2)]
            for k in range(2):
                nc.sync.dma_start(w_fin[k][:], d_fin[k])
            cst = constp.tile([128, 64], F32)
            nc.sync.dma_start(cst[:], d_consts[:])
            w_m1b32 = constp.tile([128, MID], F32)
            nc.sync.dma_start(w_m1b32[:], d_m1b32[:])
            w_m1c32 = constp.tile([128, MID], F32)
            nc.sync.dma_start(w_m1c32[:], d_m1c32[:])
            w_m232 = [constp.tile([128, MID], F32, name=f"m2w32{k}", tag=f"m2w32{k}") for k in range(2)]
            for k in range(2):
                nc.sync.dma_start(w_m232[k][:], d_m232[k])
            w_m332 = [constp.tile([128, MID], F32, name=f"m3w32{k}", tag=f"m3w32{k}") for k in range(2)]
            for k in range(2):
                nc.sync.dma_start(w_m332[k][:], d_m332[k])
            w_fin32 = [constp.tile([128, N_ACT], F32, name=f"finw32{k}", tag=f"finw32{k}") for k in range(2)]
            for k in range(2):
                nc.sync.dma_start(w_fin32[k][:], d_fin32[k])

            # const column map (see host packing below)
            col_fc1b = 0          # fc1_b chunk mc at col 0+mc
            col_brz = 2           # b_ih+b_hh rows 0:512 -> cols 2..5
            col_bin = 6           # b_ih rows 512:768 -> cols 6,7
            col_bhn = 8           # b_hh rows 512:768 -> cols 8,9
            col_m2b = 10          # m2_b cols 10,11
            col_m3b = 12          # m3_b cols 12,13
            col_finb_s1 = 14      # (srm1/sr)*fin_b per step s -> col 14+s (rows 0:32)
            col_m1b = 24          # m1_bias_eff per step s -> cols 24+2s, 25+2s

            # ---------- persistent activation tiles
            xa_t = [persist.tile([N_ACT, UN], F32, name=f"xa{u}", tag=f"xa{u}") for u in range(N_UNITS)]
            h_a = persist.tile([128, BC], F32)   # h[0:128]
            h_b = persist.tile([128, BC], F32)   # h[128:256]
            h_r = [persist.tile([128, BC], F32R, name=f"hr{k}", tag=f"hr{k}") for k in range(2)]
            for u in range(N_UNITS):
                nc.sync.dma_start(xa_t[u][:], d_xinit[:, u * UN:(u + 1) * UN])

            hid = [persist.tile([128, BC], F32, tag=f"hid{k}") for k in range(2)]
            for k in range(2):
                nc.sync.dma_start(hid[k][:], d_hid[k])
            inp = persist.tile([IN_DIM, BC], F32)
            nc.sync.dma_start(inp[:], d_in[:])

            # ================= GRU phase =================
            for u in range(N_UNITS):
                cs = slice(u * UN, (u + 1) * UN)
                # fc1 + relu -> x [256, UN] as 2 chunks
                x_sb = [work.tile([128, UN], F32, tag=f"xrelu{mc}") for mc in range(2)]
                for mc in range(2):
                    pz = ps.tile([128, UN], F32, tag="zps")
                    for ns in range(2):
                        nsl = slice(ns * 512, (ns + 1) * 512)
                        nc.tensor.matmul(
                            pz[:, nsl],
                            w_fc1[:, mc * 128:(mc + 1) * 128],
                            inp[:, u * UN + ns * 512:u * UN + (ns + 1) * 512],
                            start=True, stop=True,
                        )
                    nc.scalar.activation(x_sb[mc][:], pz[:], RELU,
                                         bias=cst[:, col_fc1b + mc:col_fc1b + mc + 1])

                # r,z gates: rows 0:512 of gi+gh accumulated
                rz = [work.tile([128, UN], F32, tag=f"rz{mc}") for mc in range(4)]
                for mc in range(4):
                    prz = ps.tile([128, UN], F32, tag="zps")
                    msl = slice(mc * 128, (mc + 1) * 128)
                    for ns in range(2):
                        nsl = slice(ns * 512, (ns + 1) * 512)
                        gsl = slice(u * UN + ns * 512, u * UN + (ns + 1) * 512)
                        for k in range(2):
                            nc.tensor.matmul(prz[:, nsl], w_wih[k][:, msl], x_sb[k][:, nsl],
                                             start=(k == 0), stop=False)
                        for k in range(2):
                            nc.tensor.matmul(prz[:, nsl], w_whh[k][:, msl], hid[k][:, gsl],
                                             start=False, stop=(k == 1))
                    nc.scalar.activation(rz[mc][:], prz[:], SIG,
                                         bias=cst[:, col_brz + mc:col_brz + mc + 1])

                # n gate
                n_sb = [work.tile([128, UN], F32, tag=f"ngate{mc}") for mc in range(2)]
                for mc in range(2):
                    msl = slice(512 + mc * 128, 512 + (mc + 1) * 128)
                    p_in = ps.tile([128, UN], F32, tag="zps")
                    p_hn = ps.tile([128, UN], F32, tag="zps")
                    for ns in range(2):
                        nsl = slice(ns * 512, (ns + 1) * 512)
                        gsl = slice(u * UN + ns * 512, u * UN + (ns + 1) * 512)
                        for k in range(2):
                            nc.tensor.matmul(p_in[:, nsl], w_wih[k][:, msl], x_sb[k][:, nsl],
                                             start=(k == 0), stop=(k == 1))
                        for k in range(2):
                            nc.tensor.matmul(p_hn[:, nsl], w_whh[k][:, msl], hid[k][:, gsl],
                                             start=(k == 0), stop=(k == 1))
                    hnb = work.tile([128, UN], F32, tag="hnb")
                    nc.vector.tensor_scalar(hnb[:], p_hn[:],
                                            cst[:, col_bhn + mc:col_bhn + mc + 1], None, ADD)
                    t1 = work.tile([128, UN], F32, tag="t1")
                    nc.gpsimd.tensor_tensor(t1[:], rz[mc][:], hnb[:], MULT)
                    s_n = work.tile([128, UN], F32, tag="sn")
                    nc.vector.scalar_tensor_tensor(
                        s_n[:], p_in[:], cst[:, col_bin + mc:col_bin + mc + 1], t1[:],
                        ADD, ADD)
                    nc.scalar.activation(n_sb[mc][:], s_n[:], TANH)

                # h' = n + z*(h-n); write into split layout + DRAM
                for mc in range(2):
                    d_t = work.tile([128, UN], F32, tag="hd")
                    nc.gpsimd.tensor_tensor(d_t[:], hid[mc][:, cs], n_sb[mc][:], mybir.AluOpType.subtract)
                    e_t = work.tile([128, UN], F32, tag="he")
                    nc.gpsimd.tensor_tensor(e_t[:], rz[2 + mc][:], d_t[:], MULT)
                    hp = work.tile([128, UN], F32, tag="hp")
                    nc.vector.tensor_tensor(hp[:], n_sb[mc][:], e_t[:], ADD)
                    # split into cat0/h_b/h_c
                    if mc == 0:
                        nc.sync.dma_start(cat0[32:128, cs], hp[0:96])
                        nc.sync.dma_start(h_b[0:32, cs], hp[96:128])
                    else:
                        nc.sync.dma_start(h_b[32:128, cs], hp[0:96])
                        nc.sync.dma_start(h_c[0:32, cs], hp[96:128])
                    nc.sync.dma_start(d_h[mc, :, cs], hp[:])

            # ================= diffusion (layer-major sweeps across units) =================
            with (
                tc.tile_pool(name="work", bufs=2) as work,
                tc.tile_pool(name="mout", bufs=2) as moutp,
                tc.tile_pool(name="small", bufs=2) as small,
                tc.tile_pool(name="noisep", bufs=2) as noisep,
            ):
              MOB = int(_os2.environ.get("MOB", "16"))
              for step, i in steps:
                  sr = float(C["sqrt_recip"][i]); srm1 = float(C["sqrt_recipm1"][i])
                  c1 = float(C["coef1"][i]); c2 = float(C["coef2"][i])
                  hp = (i >= 8)

                  def mlp_layer_u(u, rhs_tiles, wk, bias_cols, tag):
                      odt = F32 if hp else F32R
                      out = [moutp.tile([128, UN], odt, name=f"mo_{tag}{mc}",
                                        tag="mo", bufs=MOB) for mc in range(2)]
                      for mc in range(2):
                          msl = slice(mc * 128, (mc + 1) * 128)
                          pz = ps.tile([128, UN], F32, tag="zps")
                          nk = len(wk)
                          for ns in range(UN // 512):
                              nsl = slice(ns * 512, (ns + 1) * 512)
                              for k, (lhsT, rhs) in enumerate(zip(wk, rhs_tiles)):
                                  nc.tensor.matmul(pz[:, nsl], lhsT[:, msl], rhs[:, nsl],
                                                   start=(k == 0), stop=(k == nk - 1))
                          bcol = bias_cols[mc]
                          s_t = work.tile([128, UN], F32, tag="sig", bufs=WB)
                          nc.scalar.activation(s_t[:], pz[:], SIG, bias=bcol)
                          y1 = work.tile([128, UN], F32, tag="y1", bufs=WB)
                          nc.vector._custom_dve(_OPA, out=y1[:], in0=s_t[:],
                                                s0=1.0, s1=SEED_C1, imm2=2.0)
                          if hp:
                              y1b = work.tile([128, UN], F32, tag="y1b", bufs=3)
                              nc.vector._custom_dve(_OPNR, out=y1b[:], in0=s_t[:],
                                                    in1=y1[:], s0=1.0, s1=0.0, imm2=2.0)
                              y1 = y1b
                          g = work.tile([128, UN], F32, tag="g", bufs=WB)
                          nc.vector._custom_dve(_OPB, out=g[:], in0=s_t[:], in1=y1[:],
                                                s0=1.0, s1=0.0, imm2=2.0)
                          zb = work.tile([128, UN], F32, tag="zb", bufs=WB)
                          nc.scalar.activation(zb[:], pz[:], IDEN, bias=bcol)
                          nc.gpsimd.tensor_tensor(out[mc][:], zb[:], g[:], MULT)
                      return out

                  noise_cur = {}
                  if i > 0:
                      for u in range(N_UNITS):
                          nz = noisep.tile([N_ACT, UN], F32, tag="noise", bufs=4)
                          nc.sync.dma_start(nz[:], d_noise[step, :, u * UN:(u + 1) * UN])
                          noise_cur[u] = nz

                  b1 = [cst[:, col_m1b + 2 * step + mc:col_m1b + 2 * step + mc + 1]
                        for mc in range(2)]
                  b2 = [cst[:, col_m2b + mc:col_m2b + mc + 1] for mc in range(2)]
                  b3 = [cst[:, col_m3b + mc:col_m3b + mc + 1] for mc in range(2)]

                  mo1 = {}
                  for u in range(N_UNITS):
                      cs = slice(u * UN, (u + 1) * UN)
                      if hp:
                          m1rhs = [xa_t[u][:], h_a[:, cs], h_b[:, cs]]
                          m1w = [w_m1a[:], w_m1b32[:], w_m1c32[:]]
                      else:
                          m1rhs = [xa_t[u][:], h_r[0][:, cs], h_r[1][:, cs]]
                          m1w = [w_m1a[:], w_m1b[:], w_m1c[:]]
                      mo1[u] = mlp_layer_u(u, m1rhs, m1w, b1, "m1")

                  mo2 = {}
                  w2s = [w_m232[0][:], w_m232[1][:]] if hp else [w_m2[0][:], w_m2[1][:]]
                  for u in range(N_UNITS):
                      mo2[u] = mlp_layer_u(u, [mo1[u][0][:], mo1[u][1][:]], w2s, b2, "m2")

                  mo3 = {}
                  w3s = [w_m332[0][:], w_m332[1][:]] if hp else [w_m3[0][:], w_m3[1][:]]
                  for u in range(N_UNITS):
                      mo3[u] = mlp_layer_u(u, [mo2[u][0][:], mo2[u][1][:]], w3s, b3, "m3")

                  for u in range(N_UNITS):
                      pf = psfin.tile([32, UN], F32, tag="fin_ps", bufs=2)
                      for ns in range(UN // 512):
                          nsl = slice(ns * 512, (ns + 1) * 512)
                          for k in range(2):
                              wfk = w_fin32[k][:] if hp else w_fin[k][:]
                              nc.tensor.matmul(pf[:, nsl], wfk, mo3[u][k][:, nsl],
                                               start=(k == 0), stop=(k == 1))
                      xa = xa_t[u][:]
                      v = small.tile([32, UN], F32, tag="epi", bufs=4)
                      nc.vector.ln_bwd_dx(v[:], xa, pf[:], srm1 / sr,
                                          cst[0:32, col_finb_s1 + step:col_finb_s1 + step + 1],
                                          c1 * sr)
                      p_t = small.tile([32, UN], F32, tag="epi", bufs=4)
                      nc.vector.tensor_scalar(p_t[:], v[:], c1 * MAX_ACTION, -c1 * MAX_ACTION,
                                              MINOP, MAXOP)
                      t_t = small.tile([32, UN], F32, tag="epi", bufs=4)
                      nc.vector.affine_then_add(t_t[:], xa, p_t[:], c2, 0.0)
                      if i > 0:
                          nc.gpsimd.tensor_tensor(xa_t[u][:], t_t[:], noise_cur[u], ADD)
                      else:
                          nc.sync.dma_start(d_q[:, u * UN:(u + 1) * UN], t_t[:])

    nc.compile()
    _BUILT["nc"] = nc
    return nc


def _host_prep(inputs):
    """Per-core input maps."""
    C = _diff_consts()
    f32 = np.float32

    inp_t = np.ascontiguousarray(inputs["inputs"].T.astype(f32))            # [128, B]
    hid_t = np.ascontiguousarray(inputs["hidden_state"].T.astype(f32))      # [256, B]
    xin_t = np.ascontiguousarray(inputs["x_init"].T.astype(f32))            # [32, B]

    # pre-scaled noise, steps 0..8 (i = 9..1)
    noise = inputs["step_noise"].astype(f32)
    scaled = np.empty((N_T - 1, N_ACT, BATCH), dtype=f32)
    for step, i in enumerate(reversed(range(1, N_T))):
        k = f32(np.exp(0.5 * C["log_var"][i]))
        scaled[step] = (noise[step] * k).T

    fc1_t = np.ascontiguousarray(inputs["fc1_w"].T.astype(f32))             # [128,256]
    wih = inputs["gru_w_ih"].astype(f32).T                                  # [256,768]
    whh = inputs["gru_w_hh"].astype(f32).T
    wih_t = np.ascontiguousarray(wih.reshape(2, 128, 3 * RNN_H))
    whh_t = np.ascontiguousarray(whh.reshape(2, 128, 3 * RNN_H))

    m1w = inputs["m1_w"].astype(f32)                                        # [256, 304]
    m1_used = np.concatenate([m1w[:, 0:N_ACT], m1w[:, N_ACT + T_DIM:]], axis=1)  # [256,288]
    m1_t = m1_used.T                                                        # [288, 256]
    m1a = np.ascontiguousarray(m1_t[0:32]); m1b = np.ascontiguousarray(m1_t[32:160])
    m1c = np.ascontiguousarray(m1_t[160:288])
    m2_t = np.ascontiguousarray(inputs["m2_w"].astype(f32).T.reshape(2, 128, MID))
    m3_t = np.ascontiguousarray(inputs["m3_w"].astype(f32).T.reshape(2, 128, MID))
    fin_t = np.ascontiguousarray(inputs["fin_w"].astype(f32).T.reshape(2, 128, N_ACT))

    # time-MLP folded into m1 bias per step
    tm1_w = inputs["tm1_w"].astype(f32); tm1_b = inputs["tm1_b"].astype(f32)
    tm2_w = inputs["tm2_w"].astype(f32); tm2_b = inputs["tm2_b"].astype(f32)
    m1_b = inputs["m1_b"].astype(f32)
    w_te = m1w[:, N_ACT:N_ACT + T_DIM]                                      # [256, 16]

    consts = np.zeros((128, 64), dtype=f32)
    fc1_b = inputs["fc1_b"].astype(f32)
    consts[:, 0] = fc1_b[0:128]; consts[:, 1] = fc1_b[128:256]
    b_ih = inputs["gru_b_ih"].astype(f32); b_hh = inputs["gru_b_hh"].astype(f32)
    brz = b_ih[0:512] + b_hh[0:512]
    for mc in range(4):
        consts[:, 2 + mc] = brz[mc * 128:(mc + 1) * 128]
    consts[:, 6] = b_ih[512:640]; consts[:, 7] = b_ih[640:768]
    consts[:, 8] = b_hh[512:640]; consts[:, 9] = b_hh[640:768]
    m2_b = inputs["m2_b"].astype(f32); m3_b = inputs["m3_b"].astype(f32)
    consts[:, 10] = m2_b[0:128]; consts[:, 11] = m2_b[128:256]
    consts[:, 12] = m3_b[0:128]; consts[:, 13] = m3_b[128:256]
    fin_b = inputs["fin_b"].astype(f32)
    for step, i in enumerate(reversed(range(N_T))):
        sr = f32(C["sqrt_recip"][i]); srm1 = f32(C["sqrt_recipm1"][i])
        consts[0:32, 14 + step] = (srm1 / sr) * fin_b
        te = _time_emb(i)
        te = _mish_np(te @ tm1_w.T + tm1_b) @ tm2_w.T + tm2_b               # [16]
        beff = m1_b + w_te @ te                                             # [256]
        consts[:, 24 + 2 * step] = beff[0:128]
        consts[:, 24 + 2 * step + 1] = beff[128:256]

    in_maps = []
    for c in range(N_CORES):
        cs = slice(c * BC, (c + 1) * BC)
        in_maps.append({
            "inp_t": np.ascontiguousarray(inp_t[:, cs]),
            "hid_t": np.ascontiguousarray(hid_t[:, cs]).reshape(2, 128, BC),
            "xinit_t": np.ascontiguousarray(xin_t[:, cs]),
            "noise_t": np.ascontiguousarray(scaled[:, :, cs]),
            "fc1_t": fc1_t, "wih_t": wih_t, "whh_t": whh_t,
            "m1a_t": m1a, "m1b_t": m1b, "m1c_t": m1c,
            "m2_t": m2_t, "m3_t": m3_t, "fin_t": fin_t,
            "m1b32_t": m1b, "m1c32_t": m1c, "m232_t": m2_t, "m332_t": m3_t,
            "fin32_t": fin_t,
            "consts": consts,
        })
    return in_maps, C


def kernel(**inputs):
    import time as _time
    nc = _build_module()
    in_maps, C = _host_prep(inputs)
    res = None
    for _attempt in range(3):
        try:
            res = run_bass_kernel_spmd(nc, in_maps, list(range(N_CORES)))
            break
        except Exception:
            if _attempt == 2:
                raise
            _time.sleep(3.0)

    q = np.empty((BATCH, N_ACT), dtype=np.float32)
    h = np.empty((BATCH, RNN_H), dtype=np.float32)
    for c, out in enumerate(res.results):
        cs = slice(c * BC, (c + 1) * BC)
        q[cs] = out["q_out"].T
        h[cs] = out["h_out"].reshape(RNN_H, BC).T

    log_var0 = np.float32(C["log_var"][0])
    q_log = np.full((BATCH, 1), log_var0, dtype=np.float32)
    nonzero_mask = np.zeros((BATCH, 1), dtype=np.float32)
    noise = inputs["step_noise"][N_T - 1].astype(np.float32)
    return (q, h, q_log, nonzero_mask, noise)


# revision 25
# speedup vs baseline: 1.0067x; 1.0067x over previous
"""Trainium2 Bass kernel for DiffusionRNNAgent sampling.

Data-parallel over 8 NeuronCores (batch 32768 -> 4096/core). Feature-major
layout on device: activations stored [features(partitions), batch(free)].
Host does transposes, schedule constants, time-MLP folding, noise pre-scaling.

mish(x) = x * tanh(softplus(x)) is computed as x * g where, with s = sigmoid(x):
    g = num/den, num = 2s - s^2 = 1 - (s-1)^2, den = 2 - num = (s-1)^2 + 1.
den lies in [1,2], so a linear seed + 2 Newton steps gives ~1e-5 accuracy.
One ACT pass (sigmoid, fused layer bias), two custom 8-slice DVE ops, one
ACT identity pass to materialize (z+b), one gpsimd multiply.
"""

import re

import numpy as np

import concourse.bass as bass
import concourse.mybir as mybir
import concourse.tile as tile
import concourse.dve_ops as dve_ops
from concourse import bacc
from concourse.dve_spec import Spec, Src0, Src1, C0, C1, C2, sq
from concourse.bass_utils import run_bass_kernel_spmd

BATCH, IN_DIM, RNN_H, N_ACT = 32768, 128, 256, 32
T_DIM, MID, N_T, MAX_ACTION = 16, 256, 10, 1.0
N_CORES = 8
BC = BATCH // N_CORES          # 4096 batch per core
import os as _os
OPC_MODE = _os.environ.get("OPC_MODE", "gpsimd")  # gpsimd | dve
N_UNITS = 4                    # units of 1024 cols
UN = BC // N_UNITS             # 1024
F32 = mybir.dt.float32


# ---------------------------------------------------------------- custom DVE ops
def _register_dve_op(name, spec, subdim=False, perf_en=None):
    for op in dve_ops.OPS:
        if op.name == name:
            return op
    probe = dve_ops.DveOp(name=name, spec=spec, subdim=subdim, uops_sha={},
                          perf_en=perf_en or {})
    dve_ops.OPS.append(probe)
    dve_ops._SUB_OPCODE_FOR_NAME[name] = dve_ops._CUSTOM_DVE_ROW_BASE + len(dve_ops.OPS) - 1
    dve_ops.CUSTOM_DVE_SPECS[name] = spec
    shas = {}
    for ver in ("v3",):
        try:
            probe.compile(ver)
            shas[ver] = probe.uops_sha.get(ver)
        except ValueError as e:
            m = re.search(rf"{ver}: ([0-9a-f]+)", str(e))
            if not m:
                raise
            shas[ver] = m.group(1)
    final = dve_ops.DveOp(name=name, spec=spec, subdim=subdim, uops_sha=shas,
                          perf_en=perf_en or {})
    dve_ops.OPS[-1] = final
    dve_ops.CUSTOM_DVE_SPECS[name] = spec
    dve_ops._COMPILE_CACHE.pop((name, "v3"), None)
    return final


def _ref_opa(in0, in1, s0, s1, imm2):
    t = in0.astype(np.float32) - s0
    w = t * t
    den = w + s0
    y0 = s1 * (imm2 - w)
    return (y0 * (imm2 - den * y0)).astype(np.float32)


def _ref_opb(in0, in1, s0, s1, imm2):
    t = in0.astype(np.float32) - s0
    w = t * t
    den = w + s0
    y2 = in1 * (imm2 - den * in1)
    return ((s0 - w) * y2).astype(np.float32)


# opA: in0 = s (sigmoid).  C0=1, C1=seed coeff, C2=2.
#   w=(s-1)^2; den=w+1; y0=C1*(2-w); y1=y0*(2-den*y0)   -> ~0.35% recip approx
_t = Src0 - C0
_w = sq(_t)
_den = _w + C0
_y0 = C1 * (C2 - _w)
MISH_RECIP_A = Spec(body=_y0 * (C2 - _den * _y0), reference=_ref_opa)

# opB: in0 = s, in1 = y1.  y2 = NR(y1); g = (1-w)*y2 = num/den
_y2 = Src1 * (C2 - _den * Src1)
MISH_RECIP_B = Spec(body=(C0 - _w) * _y2, reference=_ref_opb)

_OPA = _register_dve_op("MISH_SEED_NR_ANT", MISH_RECIP_A, perf_en={"v3": True})
_OPB = _register_dve_op("MISH_NR_MUL_ANT", MISH_RECIP_B)

SEED_C1 = 2.0 / (2.0 + 2.25)   # minimax linear seed for 1/d on [1,2]


# ---------------------------------------------------------------- host constants
def _cosine_betas(T, s=0.008):
    steps = T + 1
    x = np.linspace(0, steps, steps)
    ac = np.cos(((x / steps) + s) / (1 + s) * np.pi * 0.5) ** 2
    ac = ac / ac[0]
    return np.clip(1.0 - ac[1:] / ac[:-1], 0.0, 0.999)


def _diff_consts():
    betas = _cosine_betas(N_T)
    alphas = 1.0 - betas
    ac = np.cumprod(alphas)
    acp = np.concatenate([[1.0], ac[:-1]])
    post_var = betas * (1.0 - acp) / (1.0 - ac)
    return dict(
        sqrt_recip=np.sqrt(1.0 / ac),
        sqrt_recipm1=np.sqrt(1.0 / ac - 1.0),
        log_var=np.log(np.maximum(post_var, 1e-20)),
        coef1=betas * np.sqrt(acp) / (1.0 - ac),
        coef2=(1.0 - acp) * np.sqrt(alphas) / (1.0 - ac),
    )


def _mish_np(x):
    return (x * np.tanh(np.log1p(np.exp(x)))).astype(np.float32)


def _time_emb(i):
    half = T_DIM // 2
    freqs = np.exp(np.arange(half, dtype=np.float32) * (-np.log(10000.0) / (half - 1)))
    ang = np.float32(i) * freqs
    return np.concatenate([np.sin(ang), np.cos(ang)]).astype(np.float32)


_BUILT = {}


def _build_module():
    """Build + finalize the Bass module once. Returns (nc, names)."""
    if "nc" in _BUILT:
        return _BUILT["nc"]

    nc = bacc.Bacc("TRN2", target_bir_lowering=False, debug=False)

    d_in = nc.dram_tensor("inp_t", [IN_DIM, BC], F32, kind="ExternalInput")
    d_hid = nc.dram_tensor("hid_t", [2, 128, BC], F32, kind="ExternalInput")
    d_xinit = nc.dram_tensor("xinit_t", [N_ACT, BC], F32, kind="ExternalInput")
    d_noise = nc.dram_tensor("noise_t", [N_T - 1, N_ACT, BC], F32, kind="ExternalInput")
    d_fc1 = nc.dram_tensor("fc1_t", [IN_DIM, RNN_H], F32, kind="ExternalInput")
    d_wih = nc.dram_tensor("wih_t", [2, 128, 3 * RNN_H], F32, kind="ExternalInput")
    d_whh = nc.dram_tensor("whh_t", [2, 128, 3 * RNN_H], F32, kind="ExternalInput")
    # m1 weights with te-columns dropped: K chunks 32(xa) / 128(h0) / 128(h1)
    d_m1a = nc.dram_tensor("m1a_t", [32, MID], F32, kind="ExternalInput")
    d_m1b = nc.dram_tensor("m1b_t", [128, MID], F32R, kind="ExternalInput")
    d_m1c = nc.dram_tensor("m1c_t", [128, MID], F32R, kind="ExternalInput")
    d_m2 = nc.dram_tensor("m2_t", [2, 128, MID], F32R, kind="ExternalInput")
    d_m3 = nc.dram_tensor("m3_t", [2, 128, MID], F32R, kind="ExternalInput")
    d_fin = nc.dram_tensor("fin_t", [2, 128, N_ACT], F32R, kind="ExternalInput")
    d_consts = nc.dram_tensor("consts", [128, 64], F32, kind="ExternalInput")
    d_m1b32 = nc.dram_tensor("m1b32_t", [128, MID], F32, kind="ExternalInput")
    d_m1c32 = nc.dram_tensor("m1c32_t", [128, MID], F32, kind="ExternalInput")
    d_m232 = nc.dram_tensor("m232_t", [2, 128, MID], F32, kind="ExternalInput")
    d_m332 = nc.dram_tensor("m332_t", [2, 128, MID], F32, kind="ExternalInput")
    d_fin32 = nc.dram_tensor("fin32_t", [2, 128, N_ACT], F32, kind="ExternalInput")

    d_q = nc.dram_tensor("q_out", [N_ACT, BC], F32, kind="ExternalOutput")
    d_h = nc.dram_tensor("h_out", [2, 128, BC], F32, kind="ExternalOutput")

    SIG = mybir.ActivationFunctionType.Sigmoid
    TANH = mybir.ActivationFunctionType.Tanh
    RELU = mybir.ActivationFunctionType.Relu
    IDEN = mybir.ActivationFunctionType.Identity
    ADD = mybir.AluOpType.add
    MULT = mybir.AluOpType.mult
    MINOP = mybir.AluOpType.min
    MAXOP = mybir.AluOpType.max

    C = _diff_consts()
    steps = list(enumerate(reversed(range(N_T))))  # (step, i): i = 9..0

    with tile.TileContext(nc) as tc:
        with (
            tc.tile_pool(name="const", bufs=1) as constp,
            tc.tile_pool(name="persist", bufs=1) as persist,
            tc.tile_pool(name="work", bufs=2) as work,
            tc.tile_pool(name="mout", bufs=2) as moutp,
            tc.tile_pool(name="small", bufs=3) as small,
            tc.tile_pool(name="noisep", bufs=2) as noisep,
            tc.tile_pool(name="ps", bufs=3, space="PSUM") as ps,
            tc.tile_pool(name="psfin", bufs=1, space="PSUM") as psfin,
        ):
            # ---------- load weights/constants
            w_fc1 = constp.tile([IN_DIM, RNN_H], F32)
            nc.sync.dma_start(w_fc1[:], d_fc1[:])
            w_wih = [constp.tile([128, 3 * RNN_H], F32, tag=f"wih{k}") for k in range(2)]
            for k in range(2):
                nc.sync.dma_start(w_wih[k][:], d_wih[k])
            w_whh = [constp.tile([128, 3 * RNN_H], F32, tag=f"whh{k}") for k in range(2)]
            for k in range(2):
                nc.sync.dma_start(w_whh[k][:], d_whh[k])
            w_m1a = constp.tile([32, MID], F32)
            nc.sync.dma_start(w_m1a[:], d_m1a[:])
            w_m1b = constp.tile([128, MID], F32R)
            nc.sync.dma_start(w_m1b[:], d_m1b[:])
            w_m1c = constp.tile([128, MID], F32R)
            nc.sync.dma_start(w_m1c[:], d_m1c[:])
            w_m2 = [constp.tile([128, MID], F32, tag=f"m2w{k}") for k in range(2)]
            for k in range(2):
                nc.sync.dma_start(w_m2[k][:], d_m2[k])
            w_m3 = [constp.tile([128, MID], F32, tag=f"m3w{k}") for k in range(2)]
            for k in range(2):
                nc.sync.dma_start(w_m3[k][:], d_m3[k])
            w_fin = [constp.tile([128, N_ACT], F32, tag=f"finw{k}") for k in range(2)]
            for k in range(2):
                nc.sync.dma_start(w_fin[k][:], d_fin[k])
            cst = constp.tile([128, 64], F32)
            nc.sync.dma_start(cst[:], d_consts[:])
            w_m1b32 = constp.tile([128, MID], F32)
            nc.sync.dma_start(w_m1b32[:], d_m1b32[:])
            w_m1c32 = constp.tile([128, MID], F32)
            nc.sync.dma_start(w_m1c32[:], d_m1c32[:])
            w_m232 = [constp.tile([128, MID], F32, name=f"m2w32{k}", tag=f"m2w32{k}") for k in range(2)]
            for k in range(2):
                nc.sync.dma_start(w_m232[k][:], d_m232[k])
            w_m332 = [constp.tile([128, MID], F32, name=f"m3w32{k}", tag=f"m3w32{k}") for k in range(2)]
            for k in range(2):
                nc.sync.dma_start(w_m332[k][:], d_m332[k])
            w_fin32 = [constp.tile([128, N_ACT], F32, name=f"finw32{k}", tag=f"finw32{k}") for k in range(2)]
            for k in range(2):
                nc.sync.dma_start(w_fin32[k][:], d_fin32[k])

            # const column map (see host packing below)
            col_fc1b = 0          # fc1_b chunk mc at col 0+mc
            col_brz = 2           # b_ih+b_hh rows 0:512 -> cols 2..5
            col_bin = 6           # b_ih rows 512:768 -> cols 6,7
            col_bhn = 8           # b_hh rows 512:768 -> cols 8,9
            col_m2b = 10          # m2_b cols 10,11
            col_m3b = 12          # m3_b cols 12,13
            col_finb_s1 = 14      # (srm1/sr)*fin_b per step s -> col 14+s (rows 0:32)
            col_m1b = 24          # m1_bias_eff per step s -> cols 24+2s, 25+2s

            # ---------- persistent activation tiles
            xa_t = [persist.tile([N_ACT, UN], F32, name=f"xa{u}", tag=f"xa{u}") for u in range(N_UNITS)]
            h_a = persist.tile([128, BC], F32)   # h[0:128]
            h_b = persist.tile([128, BC], F32)   # h[128:256]
            h_r = [persist.tile([128, BC], F32R, name=f"hr{k}", tag=f"hr{k}") for k in range(2)]
            for u in range(N_UNITS):
                nc.sync.dma_start(xa_t[u][:], d_xinit[:, u * UN:(u + 1) * UN])

            hid = [persist.tile([128, BC], F32, tag=f"hid{k}") for k in range(2)]
            for k in range(2):
                nc.sync.dma_start(hid[k][:], d_hid[k])
            inp = persist.tile([IN_DIM, BC], F32)
            nc.sync.dma_start(inp[:], d_in[:])

            # ================= GRU phase =================
            for u in range(N_UNITS):
                cs = slice(u * UN, (u + 1) * UN)
                # fc1 + relu -> x [256, UN] as 2 chunks
                x_sb = [work.tile([128, UN], F32, tag=f"xrelu{mc}") for mc in range(2)]
                for mc in range(2):
                    pz = ps.tile([128, UN], F32, tag="zps")
                    for ns in range(2):
                        nsl = slice(ns * 512, (ns + 1) * 512)
                        nc.tensor.matmul(
                            pz[:, nsl],
                            w_fc1[:, mc * 128:(mc + 1) * 128],
                            inp[:, u * UN + ns * 512:u * UN + (ns + 1) * 512],
                            start=True, stop=True,
                        )
                    nc.scalar.activation(x_sb[mc][:], pz[:], RELU,
                                         bias=cst[:, col_fc1b + mc:col_fc1b + mc + 1])

                # r,z gates: rows 0:512 of gi+gh accumulated
                rz = [work.tile([128, UN], F32, tag=f"rz{mc}") for mc in range(4)]
                for mc in range(4):
                    prz = ps.tile([128, UN], F32, tag="zps")
                    msl = slice(mc * 128, (mc + 1) * 128)
                    for ns in range(2):
                        nsl = slice(ns * 512, (ns + 1) * 512)
                        gsl = slice(u * UN + ns * 512, u * UN + (ns + 1) * 512)
                        for k in range(2):
                            nc.tensor.matmul(prz[:, nsl], w_wih[k][:, msl], x_sb[k][:, nsl],
                                             start=(k == 0), stop=False)
                        for k in range(2):
                            nc.tensor.matmul(prz[:, nsl], w_whh[k][:, msl], hid[k][:, gsl],
                                             start=False, stop=(k == 1))
                    nc.scalar.activation(rz[mc][:], prz[:], SIG,
                                         bias=cst[:, col_brz + mc:col_brz + mc + 1])

                # n gate
                n_sb = [work.tile([128, UN], F32, tag=f"ngate{mc}") for mc in range(2)]
                for mc in range(2):
                    msl = slice(512 + mc * 128, 512 + (mc + 1) * 128)
                    p_in = ps.tile([128, UN], F32, tag="zps")
                    p_hn = ps.tile([128, UN], F32, tag="zps")
                    for ns in range(2):
                        nsl = slice(ns * 512, (ns + 1) * 512)
                        gsl = slice(u * UN + ns * 512, u * UN + (ns + 1) * 512)
                        for k in range(2):
                            nc.tensor.matmul(p_in[:, nsl], w_wih[k][:, msl], x_sb[k][:, nsl],
                                             start=(k == 0), stop=(k == 1))
                        for k in range(2):
                            nc.tensor.matmul(p_hn[:, nsl], w_whh[k][:, msl], hid[k][:, gsl],
                                             start=(k == 0), stop=(k == 1))
                    hnb = work.tile([128, UN], F32, tag="hnb")
                    nc.vector.tensor_scalar(hnb[:], p_hn[:],
                                            cst[:, col_bhn + mc:col_bhn + mc + 1], None, ADD)
                    t1 = work.tile([128, UN], F32, tag="t1")
                    nc.gpsimd.tensor_tensor(t1[:], rz[mc][:], hnb[:], MULT)
                    s_n = work.tile([128, UN], F32, tag="sn")
                    nc.vector.scalar_tensor_tensor(
                        s_n[:], p_in[:], cst[:, col_bin + mc:col_bin + mc + 1], t1[:],
                        ADD, ADD)
                    nc.scalar.activation(n_sb[mc][:], s_n[:], TANH)

                # h' = n + z*(h-n); write into split layout + DRAM
                for mc in range(2):
                    d_t = work.tile([128, UN], F32, tag="hd")
                    nc.gpsimd.tensor_tensor(d_t[:], hid[mc][:, cs], n_sb[mc][:], mybir.AluOpType.subtract)
                    e_t = work.tile([128, UN], F32, tag="he")
                    nc.gpsimd.tensor_tensor(e_t[:], rz[2 + mc][:], d_t[:], MULT)
                    hp = work.tile([128, UN], F32, tag="hp")
                    nc.vector.tensor_tensor(hp[:], n_sb[mc][:], e_t[:], ADD)
                    # split into cat0/h_b/h_c
                    if mc == 0:
                        nc.sync.dma_start(cat0[32:128, cs], hp[0:96])
                        nc.sync.dma_start(h_b[0:32, cs], hp[96:128])
                    else:
                        nc.sync.dma_start(h_b[32:128, cs], hp[0:96])
                        nc.sync.dma_start(h_c[0:32, cs], hp[96:128])
                    nc.sync.dma_start(d_h[mc, :, cs], hp[:])

            # ================= diffusion (layer-major sweeps across units) =================
            with (
                tc.tile_pool(name="work", bufs=2) as work,
                tc.tile_pool(name="mout", bufs=2) as moutp,
                tc.tile_pool(name="small", bufs=2) as small,
                tc.tile_pool(name="noisep", bufs=2) as noisep,
            ):
              MOB = int(_os2.environ.get("MOB", "16"))
              for step, i in steps:
                  sr = float(C["sqrt_recip"][i]); srm1 = float(C["sqrt_recipm1"][i])
                  c1 = float(C["coef1"][i]); c2 = float(C["coef2"][i])
                  hp = (i >= 8)

                  def mlp_layer_u(u, rhs_tiles, wk, bias_cols, tag):
                      odt = F32 if hp else F32R
                      out = [moutp.tile([128, UN], odt, name=f"mo_{tag}{mc}",
                                        tag="mo", bufs=MOB) for mc in range(2)]
                      for mc in range(2):
                          msl = slice(mc * 128, (mc + 1) * 128)
                          pz = ps.tile([128, UN], F32, tag="zps")
                          nk = len(wk)
                          for ns in range(UN // 512):
                              nsl = slice(ns * 512, (ns + 1) * 512)
                              for k, (lhsT, rhs) in enumerate(zip(wk, rhs_tiles)):
                                  nc.tensor.matmul(pz[:, nsl], lhsT[:, msl], rhs[:, nsl],
                                                   start=(k == 0), stop=(k == nk - 1))
                          bcol = bias_cols[mc]
                          s_t = work.tile([128, UN], F32, tag="sig", bufs=WB)
                          nc.scalar.activation(s_t[:], pz[:], SIG, bias=bcol)
                          y1 = work.tile([128, UN], F32, tag="y1", bufs=WB)
                          nc.vector._custom_dve(_OPA, out=y1[:], in0=s_t[:],
                                                s0=1.0, s1=SEED_C1, imm2=2.0)
                          g = work.tile([128, UN], F32, tag="g", bufs=WB)
                          nc.vector._custom_dve(_OPB, out=g[:], in0=s_t[:], in1=y1[:],
                                                s0=1.0, s1=0.0, imm2=2.0)
                          zb = work.tile([128, UN], F32, tag="zb", bufs=WB)
                          nc.scalar.activation(zb[:], pz[:], IDEN, bias=bcol)
                          nc.gpsimd.tensor_tensor(out[mc][:], zb[:], g[:], MULT)
                      return out

                  noise_cur = {}
                  if i > 0:
                      for u in range(N_UNITS):
                          nz = noisep.tile([N_ACT, UN], F32, tag="noise", bufs=4)
                          nc.sync.dma_start(nz[:], d_noise[step, :, u * UN:(u + 1) * UN])
                          noise_cur[u] = nz

                  b1 = [cst[:, col_m1b + 2 * step + mc:col_m1b + 2 * step + mc + 1]
                        for mc in range(2)]
                  b2 = [cst[:, col_m2b + mc:col_m2b + mc + 1] for mc in range(2)]
                  b3 = [cst[:, col_m3b + mc:col_m3b + mc + 1] for mc in range(2)]

                  mo1 = {}
                  for u in range(N_UNITS):
                      cs = slice(u * UN, (u + 1) * UN)
                      if hp:
                          m1rhs = [xa_t[u][:], h_a[:, cs], h_b[:, cs]]
                          m1w = [w_m1a[:], w_m1b32[:], w_m1c32[:]]
                      else:
                          m1rhs = [xa_t[u][:], h_r[0][:, cs], h_r[1][:, cs]]
                          m1w = [w_m1a[:], w_m1b[:], w_m1c[:]]
                      mo1[u] = mlp_layer_u(u, m1rhs, m1w, b1, "m1")

                  mo2 = {}
                  w2s = [w_m232[0][:], w_m232[1][:]] if hp else [w_m2[0][:], w_m2[1][:]]
                  for u in range(N_UNITS):
                      mo2[u] = mlp_layer_u(u, [mo1[u][0][:], mo1[u][1][:]], w2s, b2, "m2")

                  mo3 = {}
                  w3s = [w_m332[0][:], w_m332[1][:]] if hp else [w_m3[0][:], w_m3[1][:]]
                  for u in range(N_UNITS):
                      mo3[u] = mlp_layer_u(u, [mo2[u][0][:], mo2[u][1][:]], w3s, b3, "m3")

                  for u in range(N_UNITS):
                      pf = psfin.tile([32, UN], F32, tag="fin_ps", bufs=2)
                      for ns in range(UN // 512):
                          nsl = slice(ns * 512, (ns + 1) * 512)
                          for k in range(2):
                              wfk = w_fin32[k][:] if hp else w_fin[k][:]
                              nc.tensor.matmul(pf[:, nsl], wfk, mo3[u][k][:, nsl],
                                               start=(k == 0), stop=(k == 1))
                      xa = xa_t[u][:]
                      v = small.tile([32, UN], F32, tag="epi", bufs=4)
                      nc.vector.ln_bwd_dx(v[:], xa, pf[:], srm1 / sr,
                                          cst[0:32, col_finb_s1 + step:col_finb_s1 + step + 1],
                                          c1 * sr)
                      p_t = small.tile([32, UN], F32, tag="epi", bufs=4)
                      nc.vector.tensor_scalar(p_t[:], v[:], c1 * MAX_ACTION, -c1 * MAX_ACTION,
                                              MINOP, MAXOP)
                      t_t = small.tile([32, UN], F32, tag="epi", bufs=4)
                      nc.vector.affine_then_add(t_t[:], xa, p_t[:], c2, 0.0)
                      if i > 0:
                          nc.gpsimd.tensor_tensor(xa_t[u][:], t_t[:], noise_cur[u], ADD)
                      else:
                          nc.sync.dma_start(d_q[:, u * UN:(u + 1) * UN], t_t[:])

    nc.compile()
    _BUILT["nc"] = nc
    return nc


def _host_prep(inputs):
    """Per-core input maps."""
    C = _diff_consts()
    f32 = np.float32

    inp_t = np.ascontiguousarray(inputs["inputs"].T.astype(f32))            # [128, B]
    hid_t = np.ascontiguousarray(inputs["hidden_state"].T.astype(f32))      # [256, B]
    xin_t = np.ascontiguousarray(inputs["x_init"].T.astype(f32))            # [32, B]

    # pre-scaled noise, steps 0..8 (i = 9..1)
    noise = inputs["step_noise"].astype(f32)
    scaled = np.empty((N_T - 1, N_ACT, BATCH), dtype=f32)
    for step, i in enumerate(reversed(range(1, N_T))):
        k = f32(np.exp(0.5 * C["log_var"][i]))
        scaled[step] = (noise[step] * k).T

    fc1_t = np.ascontiguousarray(inputs["fc1_w"].T.astype(f32))             # [128,256]
    wih = inputs["gru_w_ih"].astype(f32).T                                  # [256,768]
    whh = inputs["gru_w_hh"].astype(f32).T
    wih_t = np.ascontiguousarray(wih.reshape(2, 128, 3 * RNN_H))
    whh_t = np.ascontiguousarray(whh.reshape(2, 128, 3 * RNN_H))

    m1w = inputs["m1_w"].astype(f32)                                        # [256, 304]
    m1_used = np.concatenate([m1w[:, 0:N_ACT], m1w[:, N_ACT + T_DIM:]], axis=1)  # [256,288]
    m1_t = m1_used.T                                                        # [288, 256]
    m1a = np.ascontiguousarray(m1_t[0:32]); m1b = np.ascontiguousarray(m1_t[32:160])
    m1c = np.ascontiguousarray(m1_t[160:288])
    m2_t = np.ascontiguousarray(inputs["m2_w"].astype(f32).T.reshape(2, 128, MID))
    m3_t = np.ascontiguousarray(inputs["m3_w"].astype(f32).T.reshape(2, 128, MID))
    fin_t = np.ascontiguousarray(inputs["fin_w"].astype(f32).T.reshape(2, 128, N_ACT))

    # time-MLP folded into m1 bias per step
    tm1_w = inputs["tm1_w"].astype(f32); tm1_b = inputs["tm1_b"].astype(f32)
    tm2_w = inputs["tm2_w"].astype(f32); tm2_b = inputs["tm2_b"].astype(f32)
    m1_b = inputs["m1_b"].astype(f32)
    w_te = m1w[:, N_ACT:N_ACT + T_DIM]                                      # [256, 16]

    consts = np.zeros((128, 64), dtype=f32)
    fc1_b = inputs["fc1_b"].astype(f32)
    consts[:, 0] = fc1_b[0:128]; consts[:, 1] = fc1_b[128:256]
    b_ih = inputs["gru_b_ih"].astype(f32); b_hh = inputs["gru_b_hh"].astype(f32)
    brz = b_ih[0:512] + b_hh[0:512]
    for mc in range(4):
        consts[:, 2 + mc] = brz[mc * 128:(mc + 1) * 128]
    consts[:, 6] = b_ih[512:640]; consts[:, 7] = b_ih[640:768]
    consts[:, 8] = b_hh[512:640]; consts[:, 9] = b_hh[640:768]
    m2_b = inputs["m2_b"].astype(f32); m3_b = inputs["m3_b"].astype(f32)
    consts[:, 10] = m2_b[0:128]; consts[:, 11] = m2_b[128:256]
    consts[:, 12] = m3_b[0:128]; consts[:, 13] = m3_b[128:256]
    fin_b = inputs["fin_b"].astype(f32)
    for step, i in enumerate(reversed(range(N_T))):
        sr = f32(C["sqrt_recip"][i]); srm1 = f32(C["sqrt_recipm1"][i])
        consts[0:32, 14 + step] = (srm1 / sr) * fin_b
        te = _time_emb(i)
        te = _mish_np(te @ tm1_w.T + tm1_b) @ tm2_w.T + tm2_b               # [16]
        beff = m1_b + w_te @ te                                             # [256]
        consts[:, 24 + 2 * step] = beff[0:128]
        consts[:, 24 + 2 * step + 1] = beff[128:256]

    in_maps = []
    for c in range(N_CORES):
        cs = slice(c * BC, (c + 1) * BC)
        in_maps.append({
            "inp_t": np.ascontiguousarray(inp_t[:, cs]),
            "hid_t": np.ascontiguousarray(hid_t[:, cs]).reshape(2, 128, BC),
            "xinit_t": np.ascontiguousarray(xin_t[:, cs]),
            "noise_t": np.ascontiguousarray(scaled[:, :, cs]),
            "fc1_t": fc1_t, "wih_t": wih_t, "whh_t": whh_t,
            "m1a_t": m1a, "m1b_t": m1b, "m1c_t": m1c,
            "m2_t": m2_t, "m3_t": m3_t, "fin_t": fin_t,
            "m1b32_t": m1b, "m1c32_t": m1c, "m232_t": m2_t, "m332_t": m3_t,
            "fin32_t": fin_t,
            "consts": consts,
        })
    return in_maps, C


def kernel(**inputs):
    import time as _time
    nc = _build_module()
    in_maps, C = _host_prep(inputs)
    res = None
    for _attempt in range(3):
        try:
            res = run_bass_kernel_spmd(nc, in_maps, list(range(N_CORES)))
            break
        except Exception:
            if _attempt == 2:
                raise
            _time.sleep(3.0)

    q = np.empty((BATCH, N_ACT), dtype=np.float32)
    h = np.empty((BATCH, RNN_H), dtype=np.float32)
    for c, out in enumerate(res.results):
        cs = slice(c * BC, (c + 1) * BC)
        q[cs] = out["q_out"].T
        h[cs] = out["h_out"].reshape(RNN_H, BC).T

    log_var0 = np.float32(C["log_var"][0])
    q_log = np.full((BATCH, 1), log_var0, dtype=np.float32)
    nonzero_mask = np.zeros((BATCH, 1), dtype=np.float32)
    noise = inputs["step_noise"][N_T - 1].astype(np.float32)
    return (q, h, q_log, nonzero_mask, noise)
